# revision 33
# baseline (speedup 1.0000x reference)
"""Causal GQA attention (paged-KV prefill) distributed over 8 TRN2 NeuronCores.

Problem: q [4096,16,128], k/v [4096,4,128] packed as B=2 sequences of S=2048,
KV-cache scatter via slot_mapping then causal attention (GQA group 4).

Sharding: 8 cores = (B=2) x (Hkv=4). Core c handles batch c//4, kv-head c%4
with its 4 query heads. No cross-core communication needed.

Per-core kernel (Bass/Tile):
  - host pre-casts shards to bf16 and pre-tiles them to the SBUF-native
    [s%128, s//128, d] layout (contiguous 4KB DMA rows, full line rate);
    v arrives padded to 129 cols with its ones column baked in
  - xbar DMA-transpose K and Q to put head_dim on partitions (split in
    halves ordered so the first chunk's operands arrive first)
  - scores^T tile [k=128, q<=512] = kT_tile.T @ qT_chunk on TensorE (bf16),
    causally trimmed: diagonal-band tiles only compute the valid query range
  - exp(scale*s) on ScalarE straight out of PSUM, one call per 2-tile
    group; every 3rd fully-causal group instead computes exp on VectorE as
    an int16 affine whose bits are bf16(exp(x)) (Schraudolph), offloading
    the otherwise-saturated ScalarE (~4e-3 extra end-to-end error)
  - causal diag blocks masked via 0/1 triangular mult on VectorE
  - out accumulation: psum_o[q=128, 129] += probT_tile.T @ [v_tile | 1],
    the 129th column accumulates the softmax denominator for free; two
    q-subblocks pack into one PSUM bank ([128, 258]). Each bank's first
    AV opens the 2KB zero region with start=True; the bank's second
    accumulator then overwrites its has_written=0 region (two interleaved
    start-groups in one bank would clear each other's has_written bits)
  - normalize: copy PSUM->SBUF early (frees the bank), then VectorE
    reciprocal + tensor_scalar, DMA out f32
  - emission is software-pipelined (scores of unit u+1 issue before the
    exp-dependent work of unit u) and q-chunks run largest-first

PSUM budget (8 banks): scores [128,1024] x3 bufs = 6, packed out
accumulators [128,258] x2 tags x1 buf = 2.
"""

import os
import sys

import numpy as np

for _p in ("/opt/trn_rl_repo",):
    if os.path.isdir(_p) and _p not in sys.path:
        sys.path.insert(0, _p)

import ml_dtypes  # noqa: E402

from concourse import bass, bacc, mybir, tile  # noqa: E402
from concourse.bass_utils import run_bass_kernel_spmd  # noqa: E402

B, S, H, HKV, D = 2, 2048, 16, 4, 128
GRP = H // HKV  # query heads per kv head
NCORES = 8
ST = S // 128  # 16 k-tiles of 128
QB = S // 512  # 4 q-chunks of 512
SCALE = 0.08838834764831845  # 1/sqrt(128)
# Schraudolph-in-bf16-bits exp on DVE: int16 bits = A16*(scale*s) + B16
# approximate bf16(exp(scale*s)) to ~3% per element. Applied to every
# DVE_EVERY-th fully-below-diagonal score group to offload the saturated
# ScalarE; softmax renormalization cancels most of the per-element error
# (measured end-to-end ~4e-3 on top of the ~3e-3 bf16 baseline).
import math as _math

A16S = (2.0**7) / _math.log(2.0) * SCALE
B16 = 127.0 * 2**7 - 366393.0 / 2**16
DVE_EVERY = 3

F32 = mybir.dt.float32
BF16 = mybir.dt.bfloat16
I16 = mybir.dt.int16

_CACHED_NC = None


def _build_graph():
    nc = bacc.Bacc(
        "TRN2", target_bir_lowering=False, debug=False, num_devices=NCORES
    )
    # host pre-tiles shards to the SBUF-native layout [s%128, s//128, d]
    # (4KB contiguous DMA rows); v arrives with its ones column baked in
    q_ext = nc.declare_dram_parameter("q", [GRP, 128, ST, D], BF16, isOutput=False)
    k_ext = nc.declare_dram_parameter("k", [128, ST, D], BF16, isOutput=False)
    v_ext = nc.declare_dram_parameter("v", [128, ST, D + 1], BF16, isOutput=False)
    tri_ext = nc.declare_dram_parameter("tri", [128, 128], BF16, isOutput=False)
    out_ext = nc.declare_dram_parameter("out", [S, GRP, D], F32, isOutput=True)

    with tile.TileContext(nc) as tc:
        with (
            tc.tile_pool(name="const", bufs=1) as constp,
            tc.tile_pool(name="stage", bufs=2) as stagep,
            tc.tile_pool(name="kv", bufs=1) as kvp,
            tc.tile_pool(name="prob", bufs=8) as probp,
            tc.tile_pool(name="osb", bufs=4) as osbp,
            tc.tile_pool(name="small", bufs=8) as smallp,
            tc.tile_pool(name="ps_s", bufs=3, space=bass.MemorySpace.PSUM) as pss,
            tc.tile_pool(name="ps_o", bufs=1, space=bass.MemorySpace.PSUM) as pso,
        ):
            # 0/1 lower-allowed mask for diagonal blocks: tri[kk, qq] = kk <= qq
            tri = constp.tile([128, 128], BF16)
            nc.sync.dma_start(tri[:], tri_ext.ap())

            # warm the exp table set while input DMAs run
            warm = smallp.tile([128, 1], F32, tag="warm")
            nc.vector.memset(warm[:], 0.0)
            nc.scalar.activation(
                warm[:], warm[:], mybir.ActivationFunctionType.Exp
            )

            # Inputs arrive bf16 pre-tiled (host does layout prep during
            # sharding), so loads are plain full-line-rate HWDGE copies and
            # only the xbar transposes (head_dim onto partitions) remain.
            # Copy->transpose xbar-mode switches serialize the DMA pool, so
            # copies and transposes are batched, not interleaved.
            kr = k_ext.ap()
            vr = v_ext.ap()
            qr = q_ext.ap()

            v_aug = kvp.tile([128, ST, 129], BF16, tag="vaug")
            k_nat = stagep.tile([128, ST, 128], BF16, tag="knat", bufs=1)
            q_nats = [None] * GRP
            q_nats[0] = stagep.tile(
                [128, ST, 128], BF16, tag="qnat_0", name="qnat", bufs=1
            )
            kT = kvp.tile([128, ST, 128], BF16, tag="kT")
            kTf = kT[:].rearrange("d st s0 -> d (st s0)")  # [128, 2048]
            qTs = [None] * GRP
            qTfs = [None] * GRP
            for h in range(GRP):
                qTs[h] = kvp.tile(
                    [128, ST, 128], BF16, tag=f"qT{h}", name="qT"
                )
                qTfs[h] = qTs[h][:].rearrange("d st s0 -> d (st s0)")

            # first chunk is (h0, qb3): it needs qT0 cols 1536.. (= q0 rows
            # 1536.. = tiles 8-15) and kT tiles 0-1 first. Load q0, k; then
            # transpose qT0-half2 + kT-half1 (unblocks compute ~9us); v and
            # the remaining transpose halves follow.
            HF = ST // 2

            def half(ap3, lo, hi):
                return ap3[:, lo:hi, :]

            nc.sync.dma_start(q_nats[0][:], qr[0])
            nc.sync.dma_start(k_nat[:], kr)
            nc.sync.dma_start_transpose(
                out=half(qTs[0][:], HF, ST), in_=half(q_nats[0][:], HF, ST)
            )
            nc.sync.dma_start_transpose(
                out=half(kT[:], 0, HF), in_=half(k_nat[:], 0, HF)
            )
            nc.sync.dma_start(v_aug[:], vr)
            nc.sync.dma_start_transpose(
                out=half(kT[:], HF, ST), in_=half(k_nat[:], HF, ST)
            )
            t_w1 = nc.sync.dma_start_transpose(
                out=half(qTs[0][:], 0, HF), in_=half(q_nats[0][:], 0, HF)
            )
            v_augf = v_aug[:].rearrange("s0 st d -> s0 (st d)")

            wave2_state = {"t_prev": t_w1}

            def emit_wave2(h):
                q_nat = stagep.tile(
                    [128, ST, 128], BF16, tag=f"qnat_{h % 2}", name="qnat", bufs=1
                )
                ld = nc.sync.dma_start(q_nat[:], qr[h])
                tile.add_dep_helper(
                    ld.ins,
                    wave2_state["t_prev"].ins,
                    reason="wave-2 load after prior transpose",
                )
                wave2_state["t_prev"] = nc.sync.dma_start_transpose(
                    out=qTs[h][:], in_=q_nat[:]
                )

            outr = out_ext.ap().rearrange(
                "(qb j s0) h d -> qb h s0 j d", j=4, s0=128
            )

            def po_slice(po, j):
                t = po[0] if j < 2 else po[1]
                off = 129 * (j % 2)
                return t[:, off : off + 129]

            def emit_scores(h, qb, g):
                """Issue the two trimmed score matmuls for k-tile pair g."""
                kbs = (2 * g, 2 * g + 1)
                trims = [max(0, kb - 4 * qb) * 128 for kb in kbs]
                widths = [512 - t for t in trims]
                offs = [0, 512]  # fixed: two tiles never share a PSUM bank
                ps = pss.tile([128, 1024], F32, tag="s", name="ps")
                for i in (0, 1):
                    kb, t, w, o = kbs[i], trims[i], widths[i], offs[i]
                    nc.tensor.matmul(
                        ps[:, o : o + w],
                        kTf[:, kb * 128 : (kb + 1) * 128],
                        qTfs[h][:, qb * 512 + t : (qb + 1) * 512],
                        start=True,
                        stop=True,
                    )
                return (ps, kbs, trims, offs, widths)

            def emit_rest(h, qb, g, po, scored):
                """exp + mask + AV accumulation for a scored group; on the
                chunk's last group also normalize + store."""
                ps, kbs, trims, offs, widths = scored
                full = kbs[1] < 4 * qb  # both tiles fully below the diagonal
                if full:
                    exp_state["ctr"] += 1
                if full and exp_state["ctr"] % DVE_EVERY == 0:
                    # offload this group's exp to DVE (Schraudolph bf16 bits)
                    i16 = probp.tile([128, 1024], I16, tag="p", name="probTi")
                    nc.vector.tensor_scalar(
                        i16[:],
                        ps[:],
                        A16S,
                        B16,
                        mybir.AluOpType.mult,
                        mybir.AluOpType.add,
                    )
                    probT = i16.bitcast(BF16)
                else:
                    probT_t = probp.tile(
                        [128, 1024], BF16, tag="p", name="probT"
                    )
                    probT = probT_t[:]
                    if widths[0] == 512:  # contiguous [0, 512 + w1)
                        nc.scalar.activation(
                            probT[:, 0 : 512 + widths[1]],
                            ps[:, 0 : 512 + widths[1]],
                            mybir.ActivationFunctionType.Exp,
                            scale=SCALE,
                        )
                    else:  # trimmed tile0 leaves a hole: two calls
                        for o, w in ((0, widths[0]), (512, widths[1])):
                            nc.scalar.activation(
                                probT[:, o : o + w],
                                ps[:, o : o + w],
                                mybir.ActivationFunctionType.Exp,
                                scale=SCALE,
                            )
                started_banks = set()
                for i in (0, 1):
                    kb, t, o = kbs[i], trims[i], offs[i]
                    j0 = t // 128
                    diag = kb >= 4 * qb
                    if diag:  # diagonal tile: mask its first q-block
                        blk = probT[:, o : o + 128]
                        nc.vector.tensor_mul(blk, blk, tri[:])
                    # masked block's AV last so it doesn't wait on the DVE
                    js = list(range(j0 + 1, 4)) + [j0] if diag else range(4)
                    for j in js:
                        qsub = 4 * qb + j
                        co = o + (j - j0) * 128
                        # The first AV (in emission order) touching each
                        # bank at kb=0 opens its zero region with start=True
                        # (clears has_written for the whole 2KB bank); the
                        # bank's other accumulator then lands on
                        # has_written=0 and overwrites. Only the bank's last
                        # AV carries stop.
                        bank = j // 2
                        start = kb == 0 and bank not in started_banks
                        if kb == 0:
                            started_banks.add(bank)
                        nc.tensor.matmul(
                            po_slice(po, j),
                            probT[:, co : co + 128],
                            v_augf[:, kb * 129 : (kb + 1) * 129],
                            start=start,
                            stop=(j % 2 == 1 and kb == qsub),
                            skip_group_check=True,
                        )
                if g == 2 * qb + 1:  # last group: normalize + store
                    # copy PSUM->SBUF first so the po banks free ASAP (the
                    # next chunk's first AV reuses them), then normalize
                    # from SBUF where DVE runs 2x
                    acc = osbp.tile([128, 2, 258], F32, tag="acc", name="acc")
                    nc.vector.tensor_copy(acc[:, 0, :], po[0][:])
                    nc.vector.tensor_copy(acc[:, 1, :], po[1][:])
                    out_sb = osbp.tile([128, 4, 128], F32, tag="out", name="osb")
                    for j in range(4):
                        aj = acc[:, j // 2, 129 * (j % 2) : 129 * (j % 2) + 129]
                        rcp = smallp.tile([128, 1], F32, tag="rcp", name="rcp")
                        nc.vector.reciprocal(rcp[:], aj[:, 128:129])
                        nc.vector.tensor_scalar_mul(
                            out_sb[:, j, :], aj[:, 0:128], rcp[:]
                        )
                    nc.sync.dma_start(outr[qb, h], out_sb[:])

            # Software-pipelined emission: issue scores(u+1) before the
            # exp-dependent work of unit u so PE never waits on ACT.
            hooks = {
                (0, 2): lambda: emit_wave2(1),
                (1, 2): lambda: emit_wave2(2),
                (2, 2): lambda: emit_wave2(3),
            }
            exp_state = {"ctr": 0}
            pending = None
            for h in range(GRP):
                for qb in (3, 2, 1, 0):  # big chunks first, small-drain tail
                    if (h, qb) in hooks:
                        hooks[(h, qb)]()
                    # packed out accumulators: bank A holds q-subblocks 0,1
                    # at cols [0,129)/[129,258); bank B holds 2,3.
                    po01 = pso.tile([128, 258], F32, tag="o01", name="po01")
                    po23 = pso.tile([128, 258], F32, tag="o23", name="po23")
                    po = (po01, po23)
                    for g in range(2 * qb + 2):
                        scored = emit_scores(h, qb, g)
                        if pending is not None:
                            emit_rest(*pending)
                        pending = (h, qb, g, po, scored)
            emit_rest(*pending)

    nc.compile()
    return nc


def _get_nc():
    global _CACHED_NC
    if _CACHED_NC is None:
        _CACHED_NC = _build_graph()
    return _CACHED_NC


def _effective_kv(kv, cache, slot):
    """Mirror reference _store_kvcache + gather: returns cache-after-scatter
    gathered at slot positions, shape [B, S, HKV, D]."""
    valid = slot >= 0
    safe = np.where(valid, slot, 0)
    cache = np.array(cache, dtype=np.float32, copy=True)
    val = np.where(valid[:, None, None], kv, cache[safe])
    cache[safe] = val
    return cache[safe.reshape(B, S)]


def _tile_sd(x):
    """[S, D] -> [128, ST, D] with row s at [s % 128, s // 128]."""
    S_, D_ = x.shape
    return np.ascontiguousarray(
        x.reshape(S_ // 128, 128, D_).transpose(1, 0, 2)
    )


def _prep_core_inputs(qb, kk, vv, tri, c):
    bf16 = ml_dtypes.bfloat16
    b, g = c // HKV, c % HKV
    q_sh = qb[b, :, g * GRP : (g + 1) * GRP, :].astype(bf16)  # [S, GRP, D]
    q_tiled = np.stack([_tile_sd(q_sh[:, h, :]) for h in range(GRP)])
    k_tiled = _tile_sd(kk[b, :, g, :].astype(bf16))
    v_sd = vv[b, :, g, :].astype(bf16)  # [S, D]
    v_pad = np.concatenate(
        [v_sd, np.ones((S, 1), dtype=bf16)], axis=1
    )  # ones col baked in
    v_tiled = _tile_sd(v_pad)
    return {"q": q_tiled, "k": k_tiled, "v": v_tiled, "tri": tri}


def kernel(q, k, v, k_cache, v_cache, slot_mapping, batch, seqlen, **_ignored):
    q = np.asarray(q, dtype=np.float32)
    k = np.asarray(k, dtype=np.float32)
    v = np.asarray(v, dtype=np.float32)
    slot = np.asarray(slot_mapping).astype(np.int64)
    assert int(batch) == B and int(seqlen) == S
    assert q.shape == (B * S, H, D)

    kk = _effective_kv(k, k_cache, slot)  # [B, S, HKV, D]
    vv = _effective_kv(v, v_cache, slot)
    qb = q.reshape(B, S, H, D)

    tri = np.triu(np.ones((128, 128), dtype=np.float32)).astype(
        ml_dtypes.bfloat16
    )

    in_maps = [
        _prep_core_inputs(qb, kk, vv, tri, c) for c in range(NCORES)
    ]

    nc = _get_nc()
    res = run_bass_kernel_spmd(nc, in_maps, core_ids=list(range(NCORES)))

    out = np.empty((B, S, H, D), dtype=np.float32)
    for c in range(NCORES):
        b, g = c // HKV, c % HKV
        out[b, :, g * GRP : (g + 1) * GRP, :] = res.results[c]["out"]
    return out.reshape(B * S, H, D)
